# revision 1
# baseline (speedup 1.0000x reference)
"""CPAB transformer kernel for Trainium2 (8 NeuronCores, SPMD).

Problem: 1D CPAB warp. points [1, 262144] f32, theta [8, 30], basis [64, 30].
reference:
    Avees = basis @ theta.T ; As = Avees.T.reshape(8*32, 1, 2)
    Trels = expm(dT*As) -> per (theta, cell): x' = A_c * x + B_c
    32 steps of: c = clip(floor(32 x), 0, 31); x = A_c x + B_c
    out[t, 0, n] = final x for theta t, point n.

Device strategy (no gather hardware on TRN2's 128-lane engines):
the per-step update G(x) = A_{c(x)} x + B_{c(x)} is piecewise affine with
fixed breakpoints t_k = k/32.  Expand exactly as

    G(x) = alpha + beta*x + sum_k [ gamma_k * relu(x - t_k)
                                    + delta_k * step(x - t_k) ]

(beta = A_0, alpha = B_0, gamma_k = A_k - A_{k-1},
 delta_k = (B_k - B_{k-1}) + gamma_k * t_k).
Since 32*x is exact in fp32, (x >= k/32) == (floor(32x) >= k) exactly, so
the expansion reproduces the reference's cell selection semantics.

One fused custom-DVE op evaluates one knot term:
    out = in1 + relu(in0 - t_k)*gamma_k + select(in0 >= t_k, delta_k, 0)
with t_k a compile-time immediate (theta-independent) and gamma/delta as
per-partition [P,1] scalars loaded from DRAM (per-core, per-theta data) --
so a single SPMD program serves all 8 thetas.

Sharding: core t computes all 262144 points for theta t (the reference
tiles points across thetas).  Per step: 1 tensor_scalar + 31 knot ops on
the DVE, [128, 2048] fp32, ping-pong buffers; 32 steps.
"""

import os
import numpy as np

NC = 32
NSTEPS = 32
N_THETA = 8
N_POINTS = 262144
P = 128
F = N_POINTS // P  # 2048

_KNOT_OP = None
_PROGRAM = None


def _register_dve_op():
    """Register the fused knot op in concourse's custom-DVE table (runtime
    registration is the documented mechanism: the uop program is written
    into the per-NEFF DVE table at compile time)."""
    global _KNOT_OP
    if _KNOT_OP is not None:
        return _KNOT_OP
    import concourse.dve_ops as dve_ops
    from concourse.dve_ops import DveOp
    from concourse.dve_spec import Spec, Src0, Src1, C0, C1, C2, Zero, relu, select
    from concourse.dve_spec import lower as dve_lower
    from concourse.dve_uop import DveOpSpec

    for op in dve_ops.OPS:
        if op.name == "CPAB_KNOT":
            _KNOT_OP = op
            return op

    def _ref(in0, in1, s0, s1, imm2):
        x = in0.astype(np.float32)
        r = np.maximum(x - np.float32(imm2), 0).astype(np.float32)
        m1 = (r * np.float32(s0)).astype(np.float32)
        m2 = np.where(x >= np.float32(imm2), np.float32(s1), np.float32(0.0))
        return ((in1.astype(np.float32) + m1).astype(np.float32) + m2).astype(
            np.float32
        )

    body = Src1 + relu(Src0 - C2) * C0 + select(Src0 >= C2, C1, Zero)
    spec = Spec(body=body, reference=_ref)
    row = dve_ops._CUSTOM_DVE_ROW_BASE + len(dve_ops.OPS)
    shas = {}
    for ver in ("v3", "v4"):
        dspec = DveOpSpec(
            name="CPAB_KNOT", opcode=row, uops=dve_lower(spec, ver=ver), rd1_en=True
        )
        shas[ver] = dspec.sha(ver)
    op = DveOp("CPAB_KNOT", spec, subdim=False, uops_sha=shas)
    dve_ops.OPS.append(op)
    dve_ops.CUSTOM_DVE_SPECS[op.name] = op.spec
    dve_ops._SUB_OPCODE_FOR_NAME[op.name] = row
    _KNOT_OP = op
    return op


def _build_program():
    """Build + compile the SPMD Bass program (once per process)."""
    global _PROGRAM
    if _PROGRAM is not None:
        return _PROGRAM
    import concourse.bacc as bacc
    import concourse.mybir as mybir
    from concourse.tile import TileContext

    knot = _register_dve_op()

    f32 = mybir.dt.float32
    nc = bacc.Bacc("TRN2", target_bir_lowering=False, debug=False, num_devices=8)
    pts = nc.dram_tensor("points", [P, F], f32, kind="ExternalInput").ap()
    consts = nc.dram_tensor("consts", [P, 64], f32, kind="ExternalInput").ap()
    out = nc.dram_tensor("out", [P, F], f32, kind="ExternalOutput").ap()

    mult = mybir.AluOpType.mult
    add = mybir.AluOpType.add

    with TileContext(nc) as tc:
        with tc.tile_pool(name="state", bufs=1) as pool:
            cbuf = pool.tile([P, 64], f32, tag="consts")
            xb = pool.tile([P, F], f32, tag="xbuf")
            yb = pool.tile([P, F], f32, tag="ybuf")
            nc.gpsimd.dma_start(cbuf[:], consts[:])
            nc.gpsimd.dma_start(xb[:], pts[:])
            cur, nxt = xb, yb
            beta_ap = cbuf[:, 62:63]
            alpha_ap = cbuf[:, 63:64]
            for _step in range(NSTEPS):
                nc.vector.tensor_scalar(
                    nxt[:], cur[:], beta_ap, alpha_ap, mult, add
                )
                for k in range(1, NC):
                    nc.vector._custom_dve(
                        knot,
                        out=nxt[:],
                        in0=cur[:],
                        in1=nxt[:],
                        s0=cbuf[:, k - 1 : k],
                        s1=cbuf[:, 30 + k : 31 + k],
                        imm2=float(k) / NC,
                    )
                cur, nxt = nxt, cur
            nc.gpsimd.dma_start(out[:], cur[:])
    nc.compile()
    _PROGRAM = nc
    return nc


def _host_tables(theta, basis):
    """Per-(theta, cell) affine maps A, B (float64), mirroring reference."""
    dT = 1.0 / NSTEPS
    Avees = basis.astype(np.float64) @ theta.astype(np.float64).T  # [64, 8]
    As = Avees.T.reshape(theta.shape[0] * NC, 2)
    a = dT * As[:, 0]
    b = dT * As[:, 1]
    small = np.abs(a) < 1e-6
    a_safe = np.where(small, 1.0, a)
    phi = np.where(small, 1.0 + 0.5 * a, np.expm1(a_safe) / a_safe)
    A = np.exp(a).reshape(theta.shape[0], NC)
    B = (b * phi).reshape(theta.shape[0], NC)
    return A, B


def _knot_consts(A, B):
    """[n_theta, 64] fp32 const rows: gamma(31), delta(31), beta, alpha."""
    n_theta = A.shape[0]
    t_knots = np.arange(1, NC, dtype=np.float64) / NC
    gam = A[:, 1:] - A[:, :-1]
    dlt = (B[:, 1:] - B[:, :-1]) + gam * t_knots[None, :]
    consts = np.zeros((n_theta, 64), dtype=np.float32)
    consts[:, 0:31] = gam.astype(np.float32)
    consts[:, 31:62] = dlt.astype(np.float32)
    consts[:, 62] = A[:, 0].astype(np.float32)
    consts[:, 63] = B[:, 0].astype(np.float32)
    return consts


def kernel(points, theta, basis):
    from concourse.bass_utils import run_bass_kernel_spmd

    points = np.asarray(points)
    theta = np.asarray(theta)
    basis = np.asarray(basis)
    n_theta = theta.shape[0]
    assert points.shape == (1, N_POINTS) and n_theta == N_THETA

    A, B = _host_tables(theta, basis)
    consts = _knot_consts(A, B)
    pts_tile = np.ascontiguousarray(
        points[0].astype(np.float32).reshape(P, F)
    )

    nc = _build_program()
    in_maps = [
        {"points": pts_tile, "consts": np.broadcast_to(consts[t], (P, 64)).copy()}
        for t in range(n_theta)
    ]
    res = run_bass_kernel_spmd(nc, in_maps, list(range(n_theta)))
    out = np.stack(
        [res.results[t]["out"].reshape(N_POINTS) for t in range(n_theta)]
    )
    return out[:, None, :].astype(np.float32)
